# revision 1
# baseline (speedup 1.0000x reference)
"""Trainium2 Bass kernel for a full attention head (QKV proj + RoPE +
causal attention + output projection), tensor-parallel over heads on 8
NeuronCores.

Sharding: each core owns 4 of the 32 heads. w_atten columns (q,k,v) and
w_proj rows are sharded per head-group; x is replicated (pre-transposed
on host to [emb, token] layout so matmuls contract over partitions).
Each core computes a partial output [B, S, EMB] (bf16); the host sums
the 8 partials (row-parallel linear unshard).

Layout/engine choices:
- All matmul operand storage is bf16 (f32 PSUM accumulate): full PE
  rate, split Ldweights, half the DMA traffic, and 2x DVE throughput on
  element-wise work.
- q/k are computed directly in transposed [dim, token] layout with
  RoPE-pair-permuted weight columns; the rotate-half companion is a
  stream_shuffle away.
- Scores are computed transposed (s[k, q]); per query group the key
  loop emits scores one tile AHEAD of the PV matmuls so the in-order PE
  queue never blocks the Activation engine. Causal narrowing: each key
  tile only covers q >= 128j, with a [128,128] triangular 0/1 mask
  multiplied into p AFTER the exp (keeping Exp off the DVE chain).
- The softmax denominator: p tiles accumulate into a bf16 [128,512]
  accumulator on DVE, then one [1,512] ones-matmul + one K=1 broadcast
  matmul per window on PE (~0.4us/pair instead of 8.5us/pair).
- y stays in SBUF between attention and the output projection; the
  output projection runs hj-outer with 8 PSUM banks so each stationary
  y tile loads once per (token tile, hj).
"""
import numpy as np

import concourse.bass as bass
import concourse.mybir as mybir
import concourse.tile as tile
from concourse import bacc, bass_utils
from concourse.bass import ts

F32 = mybir.dt.float32
BF16 = mybir.dt.bfloat16

# Problem constants (hardcoded per contract)
B = 2
S = 2048
EMB = 4096
NH = 32
HD = 128
N_CORES = 8
H_LOC = NH // N_CORES          # heads per core = 4
FLOC = H_LOC * HD              # per-core head dims = 512
INV_SQRT_HD = 1.0 / float(np.sqrt(HD))
NEG = -1.0e9

TCH = 512                      # phase-A token chunk
NCH = B * S // TCH             # 8 chunks
NE = EMB // 128                # 32 e-tiles
NEX = 8                        # e-tiles per x SBUF tile
NW = 16                        # e-tiles per w SBUF tile
NQG = S // 512                 # 4 query groups per (b,h)
NKT = S // 128                 # 16 key tiles per (b,h)
NTT = S // 128                 # 16 token tiles

SHUF_MASK = list(range(16, 32)) + list(range(0, 16))


def _rope_perm():
    perm = np.zeros(HD, dtype=np.int64)
    for q in range(4):
        for r in range(16):
            perm[32 * q + r] = 2 * (16 * q + r)
            perm[32 * q + 16 + r] = 2 * (16 * q + r) + 1
    return perm


def host_prep(x, w_atten, w_proj, freqs_cos, freqs_sin):
    import ml_dtypes
    NPBF16 = ml_dtypes.bfloat16
    perm = _rope_perm()
    # x packed [B, 128, NE, S] so each phase-A chunk loads with 1KB runs
    xp = np.ascontiguousarray(
        x.transpose(0, 2, 1).reshape(B, NE, 128, S).transpose(0, 2, 1, 3)
    ).astype(NPBF16)                                          # [B,128,NE,S]

    cs = np.zeros((HD, S), dtype=np.float32)
    ss = np.zeros((HD, S), dtype=np.float32)
    cosT = freqs_cos.T
    sinT = freqs_sin.T
    for q in range(4):
        for r in range(16):
            i = 16 * q + r
            cs[32 * q + r] = cosT[i]
            cs[32 * q + 16 + r] = cosT[i]
            ss[32 * q + r] = -sinT[i]
            ss[32 * q + 16 + r] = sinT[i]

    # multiplicative triangular mask for the first 128 columns of each key
    # tile's strip: within tile j, key = 128j + p, q = 128j + c -> causal
    # iff c >= p (applied to p AFTER exp, keeping the Exp critical path
    # free of DVE dependencies)
    tri = np.where(np.arange(128)[None, :] >= np.arange(128)[:, None],
                   1.0, 0.0)

    shared = {
        "xt": xp,
        "cs": np.ascontiguousarray(cs).astype(NPBF16),
        "ss": np.ascontiguousarray(ss).astype(NPBF16),
        "tri": np.ascontiguousarray(tri).astype(NPBF16),
        "ones_col": np.ones((128, 1), dtype=NPBF16),
        "ones_row": np.ones((1, 128), dtype=np.float32),
    }
    per_core = []

    def pack_w(w):
        # [EMB, FLOC] -> [128, NE*FLOC] with w_p[p, e*FLOC+f] = w[128e+p, f]
        return np.ascontiguousarray(
            w.reshape(NE, 128, FLOC).transpose(1, 0, 2).reshape(128, NE * FLOC)
        ).astype(NPBF16)

    for c in range(N_CORES):
        h0 = c * H_LOC
        wq = np.empty((EMB, FLOC), dtype=np.float32)
        wk = np.empty((EMB, FLOC), dtype=np.float32)
        for j in range(H_LOC):
            qcols = (h0 + j) * HD + perm
            wq[:, j * HD:(j + 1) * HD] = w_atten[:, qcols]
            wk[:, j * HD:(j + 1) * HD] = w_atten[:, EMB + qcols]
        wv = w_atten[:, 2 * EMB + h0 * HD: 2 * EMB + (h0 + H_LOC) * HD]
        wp = w_proj[h0 * HD:(h0 + H_LOC) * HD, :]
        per_core.append({
            "wq": pack_w(wq),
            "wk": pack_w(wk),
            "wv": pack_w(np.ascontiguousarray(wv)),
            "wp": np.ascontiguousarray(wp).astype(NPBF16),
        })
    return shared, per_core


def build_nc(reps=1):
    nc = bacc.Bacc("TRN2", target_bir_lowering=False, debug=False)

    xt = nc.dram_tensor("xt", [B, 128, NE, S], BF16, kind="ExternalInput")
    wq = nc.dram_tensor("wq", [128, NE * FLOC], BF16, kind="ExternalInput")
    wk = nc.dram_tensor("wk", [128, NE * FLOC], BF16, kind="ExternalInput")
    wv = nc.dram_tensor("wv", [128, NE * FLOC], BF16, kind="ExternalInput")
    wp = nc.dram_tensor("wp", [FLOC, EMB], BF16, kind="ExternalInput")
    cs = nc.dram_tensor("cs", [128, S], BF16, kind="ExternalInput")
    ss_t = nc.dram_tensor("ss", [128, S], BF16, kind="ExternalInput")
    tri = nc.dram_tensor("tri", [128, 128], BF16, kind="ExternalInput")
    F32R = mybir.dt.float32r
    ones_col = nc.dram_tensor("ones_col", [128, 1], BF16, kind="ExternalInput")
    ones_row = nc.dram_tensor("ones_row", [1, 128], F32R, kind="ExternalInput")
    out = nc.dram_tensor("out", [B, S, EMB], BF16, kind="ExternalOutput")

    qt_d = [nc.dram_tensor(f"qt_d{b}", [FLOC, S], BF16, kind="Internal")
            for b in range(B)]
    kt_d = [nc.dram_tensor(f"kt_d{b}", [FLOC, S], BF16, kind="Internal")
            for b in range(B)]
    v_d = [nc.dram_tensor(f"v_d{b}", [S, FLOC], BF16, kind="Internal")
           for b in range(B)]

    with tile.TileContext(nc) as tc, \
         nc.allow_low_precision(reason="bf16 storage everywhere is within "
                                "this problem's error budget (measured "
                                "~7e-3 vs 2e-2 gate); PSUM accumulate "
                                "stays f32"):
      for rep in range(reps):
        # B-phase q/k/v input pool: opened before the A pools (pools are
        # a strict stack) so the first pairs' loads can overlap A's tail
        pb_ctx = tc.tile_pool(name=f"pb{rep}", bufs=2)
        pb = pb_ctx.__enter__()
        # ================= PHASE A: qkv projection + rope =============
        paw_ctx = tc.tile_pool(name=f"pa_w{rep}", bufs=1)
        paw = paw_ctx.__enter__()
        pacs_ctx = tc.tile_pool(name=f"pa_cs{rep}", bufs=1)
        pacs = pacs_ctx.__enter__()
        pax_ctx = tc.tile_pool(name=f"pa_x{rep}", bufs=2)
        pax = pax_ctx.__enter__()
        pat_ctx = tc.tile_pool(name=f"pa_t{rep}", bufs=2)
        pat = pat_ctx.__enter__()
        paps_ctx = tc.tile_pool(name=f"pa_ps{rep}", bufs=2, space="PSUM")
        paps = paps_ctx.__enter__()
        papsv_ctx = tc.tile_pool(name=f"pa_psv{rep}", bufs=1, space="PSUM")
        papsv = papsv_ctx.__enter__()

        # ---- weights + rope tables: loaded once per rep ----
        w_sb = {}

        def load_w(wnm, wt, wi, split=1):
            t_ = paw.tile([128, NW * FLOC], BF16, tag=f"{wnm}{wi}",
                          name=f"{wnm}{wi}_{rep}")
            n = NW * FLOC // split
            for s_ in range(split):
                nc.scalar.dma_start(
                    t_[:, s_ * n:(s_ + 1) * n],
                    wt.ap()[:, wi * NW * FLOC + s_ * n:
                            wi * NW * FLOC + (s_ + 1) * n])
            w_sb[(wnm, wi)] = t_

        cs_sb = pacs.tile([128, S], BF16, tag="cs")
        ss_sb = pacs.tile([128, S], BF16, tag="ss")

        def w_slice(wnm, e, lo, ln):
            t_ = w_sb[(wnm, e // NW)]
            base = (e % NW) * FLOC
            return t_[:, base + lo: base + lo + ln]

        qkv_sb = {}

        def load_pair(b, h):
            f0 = h * 128
            q_sb = pb.tile([128, S], BF16, tag="q", name=f"q_{b}_{h}_{rep}")
            k_sb = pb.tile([128, S], BF16, tag="k", name=f"k_{b}_{h}_{rep}")
            v_sb = pb.tile([128, S], BF16, tag="v", name=f"v_{b}_{h}_{rep}")
            nc.sync.dma_start(q_sb[:], qt_d[b].ap()[f0:f0 + 128, :])
            nc.sync.dma_start(k_sb[:], kt_d[b].ap()[f0:f0 + 128, :])
            nc.sync.dma_start(
                v_sb[:].rearrange("p (j f) -> p j f", j=NKT),
                v_d[b].ap()[:, f0:f0 + 128].rearrange(
                    "(j p) f -> p j f", p=128))
            qkv_sb[(b, h)] = (q_sb, k_sb, v_sb)

        for ch in range(NCH):
            b, s0 = ch // (S // TCH), (ch % (S // TCH)) * TCH
            x_parts = [None] * (NE // NEX)

            def load_x(xi, split=1):
                xp_t = pax.tile([128, NEX * TCH], BF16, tag=f"x{xi}",
                                name=f"x{xi}_{ch}_{rep}")
                n = NEX // split
                for s_ in range(split):
                    nc.sync.dma_start(
                        xp_t[:, s_ * n * TCH:(s_ + 1) * n * TCH].rearrange(
                            "p (e t) -> p e t", e=n),
                        xt.ap()[b, :, xi * NEX + s_ * n:
                                xi * NEX + (s_ + 1) * n, s0:s0 + TCH])
                x_parts[xi] = xp_t

            if ch == 0:
                # first chunk: interleave fine-grained x / w loads so the
                # first matmuls start as early as possible
                load_x(0, split=4)
                load_w("wq", wq, 0, split=4)
                load_x(1, split=2)
                load_w("wq", wq, 1, split=2)
                nc.scalar.dma_start(cs_sb[:], cs.ap()[:])
                load_x(2)
                load_w("wk", wk, 0)
                load_w("wk", wk, 1)
                load_x(3)
                load_w("wv", wv, 0)
                load_w("wv", wv, 1)
                nc.scalar.dma_start(ss_sb[:], ss_t.ap()[:])
            else:
                for xi in range(4):
                    load_x(xi)

            def x_slice(e, lo, ln):
                t_ = x_parts[e // NEX]
                base = (e % NEX) * TCH
                return t_[:, base + lo: base + lo + ln]

            # ---- q/k (transposed layout + rope) ----
            for fi in range(2 * H_LOC):
                wnm = "wq" if fi < H_LOC else "wk"
                f0 = (fi % H_LOC) * 128
                ps = paps.tile([128, TCH], F32, tag="qk_ps")
                for e in range(NE):
                    nc.tensor.matmul(
                        ps[:],
                        w_slice(wnm, e, f0, 128),
                        x_slice(e, 0, TCH),
                        start=(e == 0), stop=(e == NE - 1))
                dst = qt_d[b] if fi < H_LOC else kt_d[b]
                raw = pat.tile([128, TCH], BF16, tag="raw")
                nc.scalar.copy(raw[:], ps[:])
                shuf = pat.tile([128, TCH], BF16, tag="shuf")
                nc.vector.stream_shuffle(shuf[:], raw[:], SHUF_MASK)
                t1 = pat.tile([128, TCH], BF16, tag="t1")
                nc.vector.tensor_mul(t1[:], raw[:], cs_sb[:, s0:s0 + TCH])
                t2 = pat.tile([128, TCH], BF16, tag="t2")
                nc.vector.tensor_mul(t2[:], shuf[:], ss_sb[:, s0:s0 + TCH])
                rope = pat.tile([128, TCH], BF16, tag="rope")
                nc.vector.tensor_add(rope[:], t1[:], t2[:])
                nc.sync.dma_start(
                    dst.ap()[f0:f0 + 128, s0:s0 + TCH], rope[:])

            # ---- v (natural layout) ----
            ps_v = {}
            for tt in range(TCH // 128):
                ps_v[tt] = papsv.tile([128, FLOC], F32, tag=f"v_ps{tt}",
                                      name=f"v_ps{tt}_{ch}_{rep}")
            for e in range(NE):
                for tt in range(TCH // 128):
                    nc.tensor.matmul(
                        ps_v[tt][:],
                        x_slice(e, tt * 128, 128),
                        w_slice("wv", e, 0, FLOC),
                        start=(e == 0), stop=(e == NE - 1))
            for tt in range(TCH // 128):
                v_out = pat.tile([128, FLOC], BF16, tag="v_out")
                nc.scalar.copy(v_out[:], ps_v[tt][:])
                tglob = s0 + tt * 128
                nc.sync.dma_start(
                    v_d[b].ap()[tglob:tglob + 128, :], v_out[:])

            if ch == 5:
                load_pair(0, 0)
            elif ch == 6:
                load_pair(0, 1)

        # phase A pools close; SBUF freed for the attention working set
        papsv_ctx.__exit__(None, None, None)
        paps_ctx.__exit__(None, None, None)
        pat_ctx.__exit__(None, None, None)
        pax_ctx.__exit__(None, None, None)
        pacs_ctx.__exit__(None, None, None)
        paw_ctx.__exit__(None, None, None)

        # ============ PHASE B: causal attention (y stays in SBUF) =====
        pcy_ctx = tc.tile_pool(name=f"pcy{rep}", bufs=1)
        pcy = pcy_ctx.__enter__()
        pcw_ctx = tc.tile_pool(name=f"pc_w{rep}", bufs=1)
        pcw = pcw_ctx.__enter__()
        yt_sb = {}
        for b in range(B):
            for h in range(H_LOC):
                yt_sb[(b, h)] = pcy.tile([128, S], BF16, tag=f"yt{b}_{h}",
                                         name=f"yt_sb{b}_{h}_{rep}")
        wp_sb = {}
        for hj in range(H_LOC):
            wp_sb[hj] = pcw.tile([128, EMB], BF16, tag=f"wp{hj}",
                                 name=f"wp_sb{hj}_{rep}")
            nc.scalar.dma_start(wp_sb[hj][:],
                              wp.ap()[hj * 128:(hj + 1) * 128, :])

        with tc.tile_pool(name=f"pb_p{rep}", bufs=3) as pbp, \
             tc.tile_pool(name=f"pb_a{rep}", bufs=2) as pba, \
             tc.tile_pool(name=f"pb_d{rep}", bufs=2) as pbd, \
             tc.tile_pool(name=f"pb_m{rep}", bufs=1) as pbm, \
             tc.tile_pool(name=f"pb_s{rep}", bufs=3, space="PSUM") as pbs, \
             tc.tile_pool(name=f"pb_y{rep}", bufs=2, space="PSUM") as pby, \
             tc.tile_pool(name=f"pb_n{rep}", bufs=1, space="PSUM") as pbn:
            tri_sb = pbm.tile([128, 128], BF16, tag="tri")
            nc.scalar.dma_start(tri_sb[:], tri.ap()[:])
            oc_sb = pbm.tile([128, 1], BF16, tag="ones_col")
            nc.scalar.dma_start(oc_sb[:], ones_col.ap()[:])
            or_sb = pbm.tile([1, 128], F32R, tag="ones_row")
            nc.scalar.dma_start(or_sb[:], ones_row.ap()[:])

            pairs = [(b, h) for b in range(B) for h in range(H_LOC)]
            for pi, (b, h) in enumerate(pairs):
                if pi + 1 < len(pairs) and pairs[pi + 1] not in qkv_sb:
                    load_pair(*pairs[pi + 1])
                q_sb, k_sb, v_sb = qkv_sb.pop((b, h))
                for g in range(NQG):
                    nj = 4 * g + 4
                    q0 = 512 * g
                    y_t = pby.tile([128, 512], F32, tag="y",
                                   name=f"y_{b}_{h}_{g}_{rep}")
                    acc = pba.tile([128, 512], BF16, tag="acc",
                                   name=f"acc_{b}_{h}_{g}_{rep}")

                    def emit_scores(j, b=b, h=h, g=g, q0=q0,
                                    q_sb=q_sb, k_sb=k_sb):
                        off = max(0, 128 * j - q0)
                        s_t = pbs.tile([128, 512], F32, tag="s",
                                       name=f"s_{b}_{h}_{g}_{j}_{rep}")
                        nc.tensor.matmul(
                            s_t[:, off:512],
                            k_sb[:, ts(j, 128)],
                            q_sb[:, q0 + off:q0 + 512],
                            start=True, stop=True)
                        p_t = pbp.tile([128, 512], BF16, tag="p",
                                       name=f"p_{b}_{h}_{g}_{j}_{rep}")
                        nc.scalar.activation(
                            p_t[:, off:512], s_t[:, off:512],
                            mybir.ActivationFunctionType.Exp,
                            scale=INV_SQRT_HD)
                        if off > 0 or j * 128 == q0:  # diagonal block
                            nc.vector.tensor_mul(
                                p_t[:, off:off + 128],
                                p_t[:, off:off + 128],
                                tri_sb[:])
                        return (off, p_t)

                    prev = emit_scores(0)
                    for j in range(nj):
                        cur = prev
                        # one-ahead score emission keeps the in-order PE
                        # queue from blocking the Activation engine
                        prev = emit_scores(j + 1) if j + 1 < nj else None
                        off, p_t = cur
                        nc.tensor.matmul(
                            y_t[:, off:512],
                            v_sb[:, ts(j, 128)],
                            p_t[:, off:512],
                            start=(j == 0), stop=(j == nj - 1))
                        # denominator accumulation (bf16, DVE)
                        if j == 0:
                            nc.vector.tensor_copy(acc[:], p_t[:])
                        else:
                            nc.vector.tensor_add(
                                acc[:, off:512], acc[:, off:512],
                                p_t[:, off:512])
                    den_ps = pbn.tile([1, 512], F32, tag="den",
                                      name=f"den_{b}_{h}_{g}_{rep}")
                    nc.tensor.matmul(den_ps[:], oc_sb[:], acc[:],
                                     start=True, stop=True)
                    recip = pbd.tile([1, 512], F32R, tag="recip")
                    nc.vector.reciprocal(recip[:], den_ps[:])
                    bc_ps = pbn.tile([128, 512], F32, tag="bc",
                                     name=f"bc_{b}_{h}_{g}_{rep}")
                    nc.tensor.matmul(bc_ps[:], or_sb[:], recip[:],
                                     start=True, stop=True)
                    bc_sb = pbd.tile([128, 512], BF16, tag="bc_sb")
                    if g % 2 == 0:
                        nc.scalar.copy(bc_sb[:], bc_ps[:])
                    else:
                        nc.vector.tensor_copy(bc_sb[:], bc_ps[:])
                    nc.vector.tensor_mul(
                        yt_sb[(b, h)][:, q0:q0 + 512],
                        y_t[:], bc_sb[:])

        # ================= PHASE C: output projection =================
        with tc.tile_pool(name=f"pc{rep}", bufs=2) as pc, \
             tc.tile_pool(name=f"pc_ps{rep}", bufs=1, space="PSUM") as pcps:
            for b in range(B):
                for tt in range(NTT):
                    o_ps = {}
                    for oc in range(EMB // 512):
                        o_ps[oc] = pcps.tile([128, 512], F32, tag=f"o_ps{oc}",
                                             name=f"o_ps{oc}_{b}_{tt}_{rep}")
                    for hj in range(H_LOC):
                        for oc in range(EMB // 512):
                            nc.tensor.matmul(
                                o_ps[oc][:],
                                yt_sb[(b, hj)][:, ts(tt, 128)],
                                wp_sb[hj][:, ts(oc, 512)],
                                start=(hj == 0), stop=(hj == H_LOC - 1))
                    o_sb = pc.tile([128, EMB], BF16, tag="o")
                    for oc in range(EMB // 512):
                        if oc % 2 == 0:
                            nc.scalar.copy(o_sb[:, ts(oc, 512)], o_ps[oc][:])
                        else:
                            nc.vector.tensor_copy(o_sb[:, ts(oc, 512)],
                                                  o_ps[oc][:])
                    nc.sync.dma_start(
                        out.ap()[b, tt * 128:(tt + 1) * 128, :], o_sb[:])
        pcw_ctx.__exit__(None, None, None)
        pcy_ctx.__exit__(None, None, None)
        pb_ctx.__exit__(None, None, None)

    nc.compile()
    return nc


_NC_CACHE = None


def kernel(x, w_atten, w_proj, freqs_cos, freqs_sin):
    global _NC_CACHE
    x = np.asarray(x, dtype=np.float32)
    w_atten = np.asarray(w_atten, dtype=np.float32)
    w_proj = np.asarray(w_proj, dtype=np.float32)
    freqs_cos = np.asarray(freqs_cos, dtype=np.float32)
    freqs_sin = np.asarray(freqs_sin, dtype=np.float32)

    shared, per_core = host_prep(x, w_atten, w_proj, freqs_cos, freqs_sin)
    if _NC_CACHE is None:
        _NC_CACHE = build_nc()
    nc = _NC_CACHE
    in_maps = [{**shared, **per_core[c]} for c in range(N_CORES)]
    res = bass_utils.run_bass_kernel_spmd(nc, in_maps, core_ids=list(range(N_CORES)))
    acc = np.zeros((B, S, EMB), dtype=np.float64)
    for c in range(N_CORES):
        acc += res.results[c]["out"].astype(np.float64)
    return acc.astype(np.float32)



# revision 2
# speedup vs baseline: 1.1258x; 1.1258x over previous
"""Trainium2 Bass kernel for a full attention head (QKV proj + RoPE +
causal attention + output projection), tensor-parallel over heads on 8
NeuronCores.  v2: strip-exp, GPSIMD softmax denominator, C-into-B
interleave, streaming chunk-0 loads.

Sharding: each core owns 4 of the 32 heads; host sums the 8 partial
outputs (row-parallel linear unshard).

v2 changes over v1 (each validated against TimelineSim):
- Phase A chunk 0 streams: matmuls are emitted per 4-e-tile unit gated
  only on that unit's x/wq quarter DMAs, so PE starts ~4us in instead of
  waiting ~26us for the full 8MB x+wq block.
- Phase B scores land in [128,1024] 2-bank PSUM strips (2 key tiles side
  by side in the free dim); ONE exp instruction covers the whole strip,
  halving Activation's 352-cycle-per-instruction overhead.
- Softmax denominator: bf16 acc tile -> gpsimd.partition_all_reduce
  (Pool engine, SBUF->SBUF broadcast sum) -> DVE reciprocal -> DVE mul.
  Removes the den/bc matmuls from the PE stream and 2 PSUM banks.
- Phase C (b=0) matmuls are interleaved one-at-a-time into phase B's
  (b=1) strip loop, filling the PE bubbles left by the Act-bound
  softmax pipeline; the wp loads are deferred into pair (0,1)/(0,2) so
  they don't jam the DMA queue at the A->B transition (v1 lost ~15us
  there: tri/v_d transfers were stuck behind 4x512KB wp loads).
"""
import numpy as np

import concourse.bass as bass
import concourse.bass_isa as bass_isa
import concourse.mybir as mybir
import concourse.tile as tile
from concourse import bacc, bass_utils
from concourse.bass import ts

F32 = mybir.dt.float32
BF16 = mybir.dt.bfloat16

B = 2
S = 2048
EMB = 4096
NH = 32
HD = 128
N_CORES = 8
H_LOC = NH // N_CORES          # 4
FLOC = H_LOC * HD              # 512
INV_SQRT_HD = 1.0 / float(np.sqrt(HD))

TCH = 512                      # phase-A token chunk
NCH = B * S // TCH             # 8
NE = EMB // 128                # 32
NEX = 8                        # e-tiles per x SBUF tile
NW = 16                        # e-tiles per w SBUF tile
NQG = S // 512                 # 4 query groups per (b,h)
NTT = S // 128                 # 16 token tiles
NOC = EMB // 512               # 8 output column slices

USE_PREDUCE = True             # Pool partition_all_reduce denominator

SHUF_MASK = list(range(16, 32)) + list(range(0, 16))


def _rope_perm():
    perm = np.zeros(HD, dtype=np.int64)
    for q in range(4):
        for r in range(16):
            perm[32 * q + r] = 2 * (16 * q + r)
            perm[32 * q + 16 + r] = 2 * (16 * q + r) + 1
    return perm


def host_prep(x, w_atten, w_proj, freqs_cos, freqs_sin):
    import ml_dtypes
    NPBF16 = ml_dtypes.bfloat16
    perm = _rope_perm()
    xp = np.ascontiguousarray(
        x.transpose(0, 2, 1).reshape(B, NE, 128, S).transpose(0, 2, 1, 3)
    ).astype(NPBF16)                                          # [B,128,NE,S]

    cs = np.zeros((HD, S), dtype=np.float32)
    ss = np.zeros((HD, S), dtype=np.float32)
    cosT = freqs_cos.T
    sinT = freqs_sin.T
    for q in range(4):
        for r in range(16):
            i = 16 * q + r
            cs[32 * q + r] = cosT[i]
            cs[32 * q + 16 + r] = cosT[i]
            ss[32 * q + r] = -sinT[i]
            ss[32 * q + 16 + r] = sinT[i]

    tri = np.where(np.arange(128)[None, :] >= np.arange(128)[:, None],
                   1.0, 0.0)

    shared = {
        "xt": xp,
        "cs": np.ascontiguousarray(cs).astype(NPBF16),
        "ss": np.ascontiguousarray(ss).astype(NPBF16),
        "tri": np.ascontiguousarray(tri).astype(NPBF16),
        "ones_col": np.ones((128, 1), dtype=NPBF16),
        "ones_row": np.ones((1, 128), dtype=np.float32),
    }
    per_core = []

    def pack_w(w):
        return np.ascontiguousarray(
            w.reshape(NE, 128, FLOC).transpose(1, 0, 2).reshape(128, NE * FLOC)
        ).astype(NPBF16)

    for c in range(N_CORES):
        h0 = c * H_LOC
        wq = np.empty((EMB, FLOC), dtype=np.float32)
        wk = np.empty((EMB, FLOC), dtype=np.float32)
        for j in range(H_LOC):
            qcols = (h0 + j) * HD + perm
            wq[:, j * HD:(j + 1) * HD] = w_atten[:, qcols]
            wk[:, j * HD:(j + 1) * HD] = w_atten[:, EMB + qcols]
        wv = w_atten[:, 2 * EMB + h0 * HD: 2 * EMB + (h0 + H_LOC) * HD]
        wp = w_proj[h0 * HD:(h0 + H_LOC) * HD, :]
        per_core.append({
            "wq": pack_w(wq),
            "wk": pack_w(wk),
            "wv": pack_w(np.ascontiguousarray(wv)),
            "wp": np.ascontiguousarray(wp).astype(NPBF16),
        })
    return shared, per_core


def build_nc(reps=1):
    nc = bacc.Bacc("TRN2", target_bir_lowering=False, debug=False)

    xt = nc.dram_tensor("xt", [B, 128, NE, S], BF16, kind="ExternalInput")
    wq = nc.dram_tensor("wq", [128, NE * FLOC], BF16, kind="ExternalInput")
    wk = nc.dram_tensor("wk", [128, NE * FLOC], BF16, kind="ExternalInput")
    wv = nc.dram_tensor("wv", [128, NE * FLOC], BF16, kind="ExternalInput")
    wp = nc.dram_tensor("wp", [FLOC, EMB], BF16, kind="ExternalInput")
    cs = nc.dram_tensor("cs", [128, S], BF16, kind="ExternalInput")
    ss_t = nc.dram_tensor("ss", [128, S], BF16, kind="ExternalInput")
    tri = nc.dram_tensor("tri", [128, 128], BF16, kind="ExternalInput")
    F32R = mybir.dt.float32r
    ones_col = nc.dram_tensor("ones_col", [128, 1], BF16, kind="ExternalInput")
    ones_row = nc.dram_tensor("ones_row", [1, 128], F32R, kind="ExternalInput")
    out = nc.dram_tensor("out", [B, S, EMB], BF16, kind="ExternalOutput")

    qt_d = [nc.dram_tensor(f"qt_d{b}", [FLOC, S], BF16, kind="Internal")
            for b in range(B)]
    kt_d = [nc.dram_tensor(f"kt_d{b}", [FLOC, S], BF16, kind="Internal")
            for b in range(B)]
    v_d = [nc.dram_tensor(f"v_d{b}", [S, FLOC], BF16, kind="Internal")
           for b in range(B)]

    with tile.TileContext(nc) as tc, \
         nc.allow_low_precision(reason="bf16 storage everywhere is within "
                                "this problem's error budget (measured "
                                "~7e-3 vs 2e-2 gate); PSUM accumulate "
                                "stays f32"):
      for rep in range(reps):
        # static tiles loaded at kernel start (tiny; avoids the v1 A->B
        # stall where tri sat behind 512KB wp loads in the DMA queue)
        pst_ctx = tc.tile_pool(name=f"pst{rep}", bufs=1)
        pst = pst_ctx.__enter__()
        tri_sb = pst.tile([128, 128], BF16, tag="tri", name=f"tri_sb_{rep}")
        nc.sync.dma_start(tri_sb[:], tri.ap()[:])
        oc_sb = pst.tile([128, 1], BF16, tag="ones_col", name=f"oc_sb_{rep}")
        nc.sync.dma_start(oc_sb[:], ones_col.ap()[:])
        or_sb = pst.tile([1, 128], F32R, tag="ones_row", name=f"or_sb_{rep}")
        nc.sync.dma_start(or_sb[:], ones_row.ap()[:])

        pb_ctx = tc.tile_pool(name=f"pb{rep}", bufs=2)
        pb = pb_ctx.__enter__()
        # ================= PHASE A: qkv projection + rope =============
        paw_ctx = tc.tile_pool(name=f"pa_w{rep}", bufs=1)
        paw = paw_ctx.__enter__()
        pacs_ctx = tc.tile_pool(name=f"pa_cs{rep}", bufs=1)
        pacs = pacs_ctx.__enter__()
        pax_ctx = tc.tile_pool(name=f"pa_x{rep}", bufs=2)
        pax = pax_ctx.__enter__()
        pat_ctx = tc.tile_pool(name=f"pa_t{rep}", bufs=2)
        pat = pat_ctx.__enter__()
        pav_ctx = tc.tile_pool(name=f"pa_v{rep}", bufs=4)
        pav = pav_ctx.__enter__()
        paps_ctx = tc.tile_pool(name=f"pa_ps{rep}", bufs=4, space="PSUM")
        paps = paps_ctx.__enter__()
        papsv_ctx = tc.tile_pool(name=f"pa_psv{rep}", bufs=1, space="PSUM")
        papsv = papsv_ctx.__enter__()

        w_sb = {}

        def load_w(wnm, wt, wi, split=1, eng=None):
            t_ = paw.tile([128, NW * FLOC], BF16, tag=f"{wnm}{wi}",
                          name=f"{wnm}{wi}_{rep}")
            n = NW * FLOC // split
            for s_ in range(split):
                (eng or nc.scalar).dma_start(
                    t_[:, s_ * n:(s_ + 1) * n],
                    wt.ap()[:, wi * NW * FLOC + s_ * n:
                            wi * NW * FLOC + (s_ + 1) * n])
            w_sb[(wnm, wi)] = t_

        cs_sb = pacs.tile([128, S], BF16, tag="cs")
        ss_sb = pacs.tile([128, S], BF16, tag="ss")

        def w_slice(wnm, e, lo, ln):
            t_ = w_sb[(wnm, e // NW)]
            base = (e % NW) * FLOC
            return t_[:, base + lo: base + lo + ln]

        qkv_sb = {}

        def load_pair(b, h):
            f0 = h * 128
            q_sb = pb.tile([128, S], BF16, tag="q", name=f"q_{b}_{h}_{rep}")
            k_sb = pb.tile([128, S], BF16, tag="k", name=f"k_{b}_{h}_{rep}")
            v_sb = pb.tile([128, S], BF16, tag="v", name=f"v_{b}_{h}_{rep}")
            nc.sync.dma_start(q_sb[:], qt_d[b].ap()[f0:f0 + 128, :])
            nc.sync.dma_start(k_sb[:], kt_d[b].ap()[f0:f0 + 128, :])
            nc.sync.dma_start(
                v_sb[:].rearrange("p (j f) -> p j f", j=NTT),
                v_d[b].ap()[:, f0:f0 + 128].rearrange(
                    "(j p) f -> p j f", p=128))
            qkv_sb[(b, h)] = (q_sb, k_sb, v_sb)

        def rope_emit(fi, ps, b, s0):
            dst = qt_d[b] if fi < H_LOC else kt_d[b]
            f0 = (fi % H_LOC) * 128
            raw = pat.tile([128, TCH], BF16, tag="raw")
            nc.scalar.copy(raw[:], ps[:])
            shuf = pat.tile([128, TCH], BF16, tag="shuf")
            nc.vector.stream_shuffle(shuf[:], raw[:], SHUF_MASK)
            t1 = pat.tile([128, TCH], BF16, tag="t1")
            nc.vector.tensor_mul(t1[:], raw[:], cs_sb[:, s0:s0 + TCH])
            t2 = pat.tile([128, TCH], BF16, tag="t2")
            nc.vector.tensor_mul(t2[:], shuf[:], ss_sb[:, s0:s0 + TCH])
            rope = pat.tile([128, TCH], BF16, tag="rope")
            nc.vector.tensor_add(rope[:], t1[:], t2[:])
            nc.sync.dma_start(
                dst.ap()[f0:f0 + 128, s0:s0 + TCH], rope[:])

        for ch in range(NCH):
            b, s0 = ch // (S // TCH), (ch % (S // TCH)) * TCH
            x_parts = [None] * (NE // NEX)

            def load_x(xi, split=1):
                xp_t = pax.tile([128, NEX * TCH], BF16, tag=f"x{xi}",
                                name=f"x{xi}_{ch}_{rep}")
                n = NEX // split
                for s_ in range(split):
                    nc.sync.dma_start(
                        xp_t[:, s_ * n * TCH:(s_ + 1) * n * TCH].rearrange(
                            "p (e t) -> p e t", e=n),
                        xt.ap()[b, :, xi * NEX + s_ * n:
                                xi * NEX + (s_ + 1) * n, s0:s0 + TCH])
                x_parts[xi] = xp_t

            def x_slice(e, lo, ln):
                t_ = x_parts[e // NEX]
                base = (e % NEX) * TCH
                return t_[:, base + lo: base + lo + ln]

            if ch == 0:
                # streaming start: interleave x halves (sync q) with wq
                # quarters (scalar q); emit matmuls per 4-e unit so PE
                # starts as soon as the first ~1.5MB lands
                for xi in range(4):
                    load_x(xi, split=2)
                load_w("wq", wq, 0, split=4)
                load_w("wq", wq, 1, split=4)
                nc.scalar.dma_start(cs_sb[:], cs.ap()[:])
                nc.scalar.dma_start(ss_sb[:], ss_t.ap()[:])
                load_w("wk", wk, 0, split=2, eng=nc.sync)
                load_w("wk", wk, 1, split=2, eng=nc.sync)
                load_w("wv", wv, 0, split=2)
                load_w("wv", wv, 1, split=2, eng=nc.sync)

                ps_q = {}
                for fi in range(H_LOC):
                    ps_q[fi] = paps.tile([128, TCH], F32, tag=f"qk_ps",
                                         name=f"qk_ps{fi}_0_{rep}")
                # q heads: 4-e units so each gates on one x half + wq qtr
                for u in range(8):
                    for fi in range(H_LOC):
                        for e in range(4 * u, 4 * u + 4):
                            nc.tensor.matmul(
                                ps_q[fi][:],
                                w_slice("wq", e, fi * 128, 128),
                                x_slice(e, 0, TCH),
                                start=(e == 0), stop=(e == NE - 1))
                for fi in range(H_LOC):
                    rope_emit(fi, ps_q[fi], b, s0)
                # k heads: 8-e units (wk halves)
                ps_k = {}
                for fi in range(H_LOC):
                    ps_k[fi] = paps.tile([128, TCH], F32, tag=f"qk_ps",
                                         name=f"qk_ps{fi}_0k_{rep}")
                for u in range(4):
                    for fi in range(H_LOC):
                        for e in range(8 * u, 8 * u + 8):
                            nc.tensor.matmul(
                                ps_k[fi][:],
                                w_slice("wk", e, fi * 128, 128),
                                x_slice(e, 0, TCH),
                                start=(e == 0), stop=(e == NE - 1))
                for fi in range(H_LOC):
                    rope_emit(fi + H_LOC, ps_k[fi], b, s0)
            else:
                for xi in range(4):
                    load_x(xi)
                # ---- q/k (transposed layout + rope) ----
                for fi in range(2 * H_LOC):
                    wnm = "wq" if fi < H_LOC else "wk"
                    f0 = (fi % H_LOC) * 128
                    ps = paps.tile([128, TCH], F32, tag="qk_ps")
                    for e in range(NE):
                        nc.tensor.matmul(
                            ps[:],
                            w_slice(wnm, e, f0, 128),
                            x_slice(e, 0, TCH),
                            start=(e == 0), stop=(e == NE - 1))
                    rope_emit(fi, ps, b, s0)

            # ---- v (natural layout) ----
            ps_v = {}
            for tt in range(TCH // 128):
                ps_v[tt] = papsv.tile([128, FLOC], F32, tag=f"v_ps{tt}",
                                      name=f"v_ps{tt}_{ch}_{rep}")
            for e in range(NE):
                for tt in range(TCH // 128):
                    nc.tensor.matmul(
                        ps_v[tt][:],
                        x_slice(e, tt * 128, 128),
                        w_slice("wv", e, 0, FLOC),
                        start=(e == 0), stop=(e == NE - 1))
            for tt in range(TCH // 128):
                v_out = pav.tile([128, FLOC], BF16, tag="v_out")
                if tt % 2 == 0:
                    nc.scalar.copy(v_out[:], ps_v[tt][:])
                else:
                    nc.vector.tensor_copy(v_out[:], ps_v[tt][:])
                tglob = s0 + tt * 128
                nc.sync.dma_start(
                    v_d[b].ap()[tglob:tglob + 128, :], v_out[:])

            if ch == 5:
                load_pair(0, 0)
            elif ch == 6:
                load_pair(0, 1)

        papsv_ctx.__exit__(None, None, None)
        paps_ctx.__exit__(None, None, None)
        pav_ctx.__exit__(None, None, None)
        pat_ctx.__exit__(None, None, None)
        pax_ctx.__exit__(None, None, None)
        pacs_ctx.__exit__(None, None, None)
        paw_ctx.__exit__(None, None, None)

        # ============ PHASE B: causal attention (+ interleaved C) =====
        pcy_ctx = tc.tile_pool(name=f"pcy{rep}", bufs=1)
        pcy = pcy_ctx.__enter__()
        pcw_ctx = tc.tile_pool(name=f"pc_w{rep}", bufs=1)
        pcw = pcw_ctx.__enter__()
        yt_sb = {}
        for b in range(B):
            for h in range(H_LOC):
                yt_sb[(b, h)] = pcy.tile([128, S], BF16, tag=f"yt{b}_{h}",
                                         name=f"yt_sb{b}_{h}_{rep}")
        wp_sb = {}
        for hj in range(H_LOC):
            wp_sb[hj] = pcw.tile([128, EMB], BF16, tag=f"wp{hj}",
                                 name=f"wp_sb{hj}_{rep}")

        def load_wp(hj):
            nc.scalar.dma_start(wp_sb[hj][:],
                                wp.ap()[hj * 128:(hj + 1) * 128, :])

        with tc.tile_pool(name=f"pb_p{rep}", bufs=3) as pbp, \
             tc.tile_pool(name=f"pb_a{rep}", bufs=2) as pba, \
             tc.tile_pool(name=f"pb_d{rep}", bufs=2) as pbd, \
             tc.tile_pool(name=f"pb_s{rep}", bufs=2, space="PSUM") as pbs, \
             tc.tile_pool(name=f"pb_y{rep}", bufs=2, space="PSUM") as pby, \
             tc.tile_pool(name=f"pb_co{rep}", bufs=2, space="PSUM") as pco, \
             tc.tile_pool(name=f"pb_cs{rep}", bufs=3) as pcs:

            # -------- interleaved C slot machinery --------
            c_state = {"slot": 0, "hj": 0, "ops": None, "done": 0}
            c_slots = [(0, tt, oc) for tt in range(NTT) for oc in range(NOC)]

            def c_step(pool):
                """Emit ONE phase-C matmul (plus copy+DMA when a slot
                completes). Returns False when all b=0 slots are done."""
                if c_state["slot"] >= len(c_slots):
                    return False
                b_, tt, oc = c_slots[c_state["slot"]]
                hj = c_state["hj"]
                if hj == 0:
                    c_state["ops"] = pool.tile(
                        [128, 512], F32, tag="o_ps",
                        name=f"o_ps_{b_}_{tt}_{oc}_{rep}")
                nc.tensor.matmul(
                    c_state["ops"][:],
                    yt_sb[(b_, hj)][:, ts(tt, 128)],
                    wp_sb[hj][:, ts(oc, 512)],
                    start=(hj == 0), stop=(hj == H_LOC - 1))
                if hj == H_LOC - 1:
                    o_sb = pcs.tile([128, 512], BF16, tag="o_sb",
                                    name=f"o_sb_{b_}_{tt}_{oc}_{rep}")
                    if c_state["slot"] % 2 == 0:
                        nc.scalar.copy(o_sb[:], c_state["ops"][:])
                    else:
                        nc.vector.tensor_copy(o_sb[:], c_state["ops"][:])
                    nc.sync.dma_start(
                        out.ap()[b_, tt * 128:(tt + 1) * 128,
                                 oc * 512:(oc + 1) * 512], o_sb[:])
                    c_state["slot"] += 1
                    c_state["hj"] = 0
                    c_state["done"] += 1
                else:
                    c_state["hj"] = hj + 1
                return True

            pairs = [(b, h) for b in range(B) for h in range(H_LOC)]
            for pi, (b, h) in enumerate(pairs):
                if pi + 1 < len(pairs) and pairs[pi + 1] not in qkv_sb:
                    load_pair(*pairs[pi + 1])
                if pi == 2:
                    load_wp(0)
                    load_wp(1)
                elif pi == 3:
                    load_wp(2)
                    load_wp(3)
                q_sb, k_sb, v_sb = qkv_sb.pop((b, h))
                interleave = b == 1
                for g in range(NQG):
                    nj = 4 * g + 4
                    ns = nj // 2
                    q0 = 512 * g
                    y_t = pby.tile([128, 512], F32, tag="y",
                                   name=f"y_{b}_{h}_{g}_{rep}")
                    acc = pba.tile([128, 512], BF16, tag="acc",
                                   name=f"acc_{b}_{h}_{g}_{rep}")

                    def emit_strip(si, b=b, h=h, g=g, q0=q0,
                                   q_sb=q_sb, k_sb=k_sb):
                        t0, t1 = 2 * si, 2 * si + 1
                        off0 = max(0, 128 * t0 - q0)
                        off1 = max(0, 128 * t1 - q0)
                        s_t = pbs.tile([128, 1024], F32, tag="s",
                                       name=f"s_{b}_{h}_{g}_{si}_{rep}")
                        nc.tensor.matmul(
                            s_t[:, off0:512],
                            k_sb[:, ts(t0, 128)],
                            q_sb[:, q0 + off0:q0 + 512],
                            start=True, stop=True)
                        nc.tensor.matmul(
                            s_t[:, 512 + off1:1024],
                            k_sb[:, ts(t1, 128)],
                            q_sb[:, q0 + off1:q0 + 512],
                            start=True, stop=True)
                        p_t = pbp.tile([128, 1024], BF16, tag="p",
                                       name=f"p_{b}_{h}_{g}_{si}_{rep}")
                        nc.scalar.activation(
                            p_t[:, off0:1024], s_t[:, off0:1024],
                            mybir.ActivationFunctionType.Exp,
                            scale=INV_SQRT_HD)
                        if 128 * t0 >= q0:   # t0 is a diagonal tile
                            nc.vector.tensor_mul(
                                p_t[:, off0:off0 + 128],
                                p_t[:, off0:off0 + 128], tri_sb[:])
                        if 128 * t1 >= q0:   # t1 is a diagonal tile
                            nc.vector.tensor_mul(
                                p_t[:, 512 + off1:512 + off1 + 128],
                                p_t[:, 512 + off1:512 + off1 + 128],
                                tri_sb[:])
                        return (off0, off1, p_t)

                    prev = emit_strip(0)
                    for si in range(ns):
                        cur = prev
                        prev = emit_strip(si + 1) if si + 1 < ns else None
                        off0, off1, p_t = cur
                        t0, t1 = 2 * si, 2 * si + 1
                        nc.tensor.matmul(
                            y_t[:, off0:512],
                            v_sb[:, ts(t0, 128)],
                            p_t[:, off0:512],
                            start=(si == 0), stop=False)
                        nc.tensor.matmul(
                            y_t[:, off1:512],
                            v_sb[:, ts(t1, 128)],
                            p_t[:, 512 + off1:1024],
                            start=False, stop=(si == ns - 1))
                        if si == 0:
                            nc.vector.tensor_copy(acc[:], p_t[:, 0:512])
                        else:
                            nc.vector.tensor_add(
                                acc[:, off0:512], acc[:, off0:512],
                                p_t[:, off0:512])
                        nc.vector.tensor_add(
                            acc[:, off1:512], acc[:, off1:512],
                            p_t[:, 512 + off1:1024])
                        if interleave:
                            c_step(pco)
                    # ---- softmax denominator + normalize ----
                    if USE_PREDUCE:
                        denb = pbd.tile([128, 512], F32, tag="den",
                                        name=f"den_{b}_{h}_{g}_{rep}")
                        nc.gpsimd.partition_all_reduce(
                            denb[:], acc[:], 128, bass_isa.ReduceOp.add)
                        recipb = pbd.tile([128, 512], F32, tag="recip",
                                          name=f"recip_{b}_{h}_{g}_{rep}")
                        nc.vector.reciprocal(recipb[:], denb[:])
                        if interleave:
                            c_step(pco)
                            c_step(pco)
                        nc.vector.tensor_mul(
                            yt_sb[(b, h)][:, q0:q0 + 512],
                            y_t[:], recipb[:])
                    else:
                        den_ps = pco.tile([1, 512], F32, tag="den1",
                                          name=f"den_{b}_{h}_{g}_{rep}")
                        nc.tensor.matmul(den_ps[:], oc_sb[:], acc[:],
                                         start=True, stop=True)
                        recip = pbd.tile([1, 512], F32R, tag="recip1")
                        nc.vector.reciprocal(recip[:], den_ps[:])
                        bc_ps = pco.tile([128, 512], F32, tag="bc",
                                         name=f"bc_{b}_{h}_{g}_{rep}")
                        nc.tensor.matmul(bc_ps[:], or_sb[:], recip[:],
                                         start=True, stop=True)
                        bc_sb = pbd.tile([128, 512], BF16, tag="bc_sb")
                        nc.scalar.copy(bc_sb[:], bc_ps[:])
                        nc.vector.tensor_mul(
                            yt_sb[(b, h)][:, q0:q0 + 512],
                            y_t[:], bc_sb[:])

            # drain any remaining interleave state mid-slot
            while c_state["hj"] != 0:
                c_step(pco)
            n_done = c_state["slot"]

        # ================= PHASE C: output projection (remainder) =====
        with tc.tile_pool(name=f"pc{rep}", bufs=2) as pc, \
             tc.tile_pool(name=f"pc_ps{rep}", bufs=1, space="PSUM") as pcps:
            rem = ([(0, tt, oc) for tt in range(NTT) for oc in range(NOC)]
                   [n_done:])
            rem += [(1, tt, oc) for tt in range(NTT) for oc in range(NOC)]
            # group remaining slots into (b, tt) blocks of contiguous oc
            blocks = {}
            for b_, tt, oc in rem:
                blocks.setdefault((b_, tt), []).append(oc)
            for (b_, tt), ocs in blocks.items():
                o_ps = {}
                for oc in ocs:
                    o_ps[oc] = pcps.tile([128, 512], F32, tag=f"o_ps{oc}",
                                         name=f"o_ps{oc}_{b_}_{tt}_{rep}")
                for hj in range(H_LOC):
                    for oc in ocs:
                        nc.tensor.matmul(
                            o_ps[oc][:],
                            yt_sb[(b_, hj)][:, ts(tt, 128)],
                            wp_sb[hj][:, ts(oc, 512)],
                            start=(hj == 0), stop=(hj == H_LOC - 1))
                o_sb = pc.tile([128, len(ocs) * 512], BF16, tag="o",
                               name=f"o_{b_}_{tt}_{rep}",
                               padded_shape=[128, EMB])
                for i, oc in enumerate(ocs):
                    if i % 2 == 0:
                        nc.scalar.copy(o_sb[:, ts(i, 512)], o_ps[oc][:])
                    else:
                        nc.vector.tensor_copy(o_sb[:, ts(i, 512)],
                                              o_ps[oc][:])
                if ocs == list(range(min(ocs), min(ocs) + len(ocs))):
                    nc.sync.dma_start(
                        out.ap()[b_, tt * 128:(tt + 1) * 128,
                                 min(ocs) * 512:(min(ocs) + len(ocs)) * 512],
                        o_sb[:, 0:len(ocs) * 512])
                else:
                    for i, oc in enumerate(ocs):
                        nc.sync.dma_start(
                            out.ap()[b_, tt * 128:(tt + 1) * 128,
                                     oc * 512:(oc + 1) * 512],
                            o_sb[:, ts(i, 512)])
        pcw_ctx.__exit__(None, None, None)
        pcy_ctx.__exit__(None, None, None)
        pb_ctx.__exit__(None, None, None)
        pst_ctx.__exit__(None, None, None)

    nc.compile()
    return nc


_NC_CACHE = None


def kernel(x, w_atten, w_proj, freqs_cos, freqs_sin):
    global _NC_CACHE
    x = np.asarray(x, dtype=np.float32)
    w_atten = np.asarray(w_atten, dtype=np.float32)
    w_proj = np.asarray(w_proj, dtype=np.float32)
    freqs_cos = np.asarray(freqs_cos, dtype=np.float32)
    freqs_sin = np.asarray(freqs_sin, dtype=np.float32)

    shared, per_core = host_prep(x, w_atten, w_proj, freqs_cos, freqs_sin)
    if _NC_CACHE is None:
        _NC_CACHE = build_nc()
    nc = _NC_CACHE
    in_maps = [{**shared, **per_core[c]} for c in range(N_CORES)]
    res = bass_utils.run_bass_kernel_spmd(nc, in_maps, core_ids=list(range(N_CORES)))
    acc = np.zeros((B, S, EMB), dtype=np.float64)
    for c in range(N_CORES):
        acc += res.results[c]["out"].astype(np.float64)
    return acc.astype(np.float32)
